# revision 1
# baseline (speedup 1.0000x reference)
"""Causal self-attention on 8 trn2 NeuronCores.

Sharding: tensor-parallel over heads (2 heads/core) for qkv-proj +
attention; AllGather of per-core attention outputs (transposed layout);
column-parallel out-projection (128 output columns/core); host concat.

Layout notes (per core, heads h0=2i, h1=2i+1):
  Q2T/K2T  [128, T]   transposed q/k, head h0 dims on partitions 0:64,
                      h1 on 64:128.
  VN       [128, 16, 65] per head: V natural k-chunks + ones column
                      (column 64) so the AV matmul's M=65 also produces
                      the softmax row-sum in psum partition 64.
  scores   S^T chunk [128 k, 512 q] = lhsT(K2T slice).T @ rhs(Q2T slice)
  E^T      exp(S/8) via ACT from psum, causal band handled by a
           host-precomputed [128,128] 0/1 band mask + prefix zeroing.
  attn     out_u^T[d, q] accumulated over k-chunks; normalized by
           broadcasting 1/rowsum along partitions with a K=1 matmul
           (ones outer product) and a DVE multiply.
"""

import numpy as np

D_MODEL = 1024
N_HEADS = 16
HEAD_DIM = 64
B = 4
T = 2048
N_CORES = 8
HPC = 2           # heads per core
TQ = 512          # q tile
KC = 128          # k chunk
GROUP = 2         # k-chunks per exp group (psum: GROUP*2 banks in scores pool)
NT = T // TQ      # t/q tiles per batch (4)
NKC = T // KC     # k chunks per batch (16)
NCC = D_MODEL // 128  # contraction chunks (8)
FLT_MAX = np.float32(3.4028235e38)
USE_F32R = True   # float32r matmuls: 4x PE throughput, ~2.5e-4 rel err
MODEL_NO_COLLECTIVE = False  # timing-analysis only: swap AG for local DMA

_PROGRAM_CACHE = {}


def _split_multi_waits(nc, max_waits=1):
    """This toolchain's walrus encodes at most one sync-wait per
    instruction; hoist excess waits onto same-engine carrier nops."""
    import concourse.mybir as mybir

    ctr = 0
    for f in nc.m.functions:
        new_blocks = []
        for bb in f.blocks:
            insts = list(bb.instructions)
            if not any(
                inst.sync_info is not None and len(inst.sync_info.on_wait) > max_waits
                for inst in insts
            ):
                new_blocks.append(bb)
                continue
            out = []
            for inst in insts:
                si = inst.sync_info
                if si is not None and len(si.on_wait) > max_waits:
                    waits = list(si.on_wait)
                    excess = waits[max_waits:]
                    while excess:
                        ctr += 1
                        nop = mybir.InstNoOp(
                            name=f"waitcarrier-{ctr}", engine=inst.engine
                        )
                        nop.sync_info = mybir.SyncInfo(
                            on_wait=excess[:max_waits], on_update=[]
                        )
                        out.append(nop)
                        excess = excess[max_waits:]
                    si.on_wait = waits[:max_waits]
                out.append(inst)
            nb = mybir.BasicBlock(
                name=bb.name,
                instructions=[],
                IsPredicated=bb.IsPredicated,
                IsExit=bb.IsExit,
                IsLoopEntry=bb.IsLoopEntry,
            )
            for i in out:
                nb.add_instruction(i)
            new_blocks.append(nb)
        f.blocks = new_blocks


def _build_program():
    import concourse.bass as bass
    import concourse.mybir as mybir
    import concourse.tile as tile
    from contextlib import ExitStack

    f32 = mybir.dt.float32
    f32r = mybir.dt.float32r if USE_F32R else mybir.dt.float32
    nc = bass.Bass()

    xT = nc.declare_dram_parameter("xT", [D_MODEL, B * T], f32, isOutput=False)
    wq = nc.declare_dram_parameter("wq", [D_MODEL, 128], f32, isOutput=False)
    wk = nc.declare_dram_parameter("wk", [D_MODEL, 128], f32, isOutput=False)
    wv = nc.declare_dram_parameter("wv", [D_MODEL, 128], f32, isOutput=False)
    wo = nc.declare_dram_parameter("wo", [D_MODEL, 128], f32, isOutput=False)
    band_in = nc.declare_dram_parameter("band", [128, 128], f32, isOutput=False)
    ident_in = nc.declare_dram_parameter("ident", [64, 64], f32, isOutput=False)
    outT = nc.declare_dram_parameter("outT", [128, B * T], f32, isOutput=True)

    attn_dram = [nc.dram_tensor(f"attn_d{b}", [128, T], f32) for b in range(B)]
    ag_out = [
        nc.dram_tensor(f"ag_out{b}", [N_CORES * 128, T], f32, addr_space="Shared")
        for b in range(B)
    ]

    with tile.TileContext(nc) as tc, ExitStack() as ctx:
        const_pool = ctx.enter_context(tc.tile_pool(name="const", bufs=1))
        qk_pool = ctx.enter_context(tc.tile_pool(name="qk", bufs=2))
        vn_pool = ctx.enter_context(tc.tile_pool(name="vn", bufs=4))
        stream_pool = ctx.enter_context(tc.tile_pool(name="stream", bufs=2))
        v2t_pool = ctx.enter_context(tc.tile_pool(name="v2t", bufs=2))
        e_pool = ctx.enter_context(tc.tile_pool(name="etile", bufs=4))
        gath_pool = ctx.enter_context(tc.tile_pool(name="gath", bufs=3))
        attnall_pool = ctx.enter_context(tc.tile_pool(name="attnall", bufs=2))
        osb_pool = ctx.enter_context(tc.tile_pool(name="osb", bufs=2))
        # psum: proj/vtr/outproj pool 2 banks, scores 2*GROUP*2 banks, av 2
        proj_ps = ctx.enter_context(tc.tile_pool(name="projps", bufs=1, space="PSUM"))
        sc_ps = ctx.enter_context(tc.tile_pool(name="scps", bufs=2, space="PSUM"))
        av_ps = ctx.enter_context(tc.tile_pool(name="avps", bufs=3, space="PSUM"))

        # ---- constants / weights ----
        band = const_pool.tile([128, 128], f32r)
        nc.sync.dma_start(out=band[:], in_=band_in[:].bitcast(f32r))
        ident = const_pool.tile([128, 64], f32r)
        nc.sync.dma_start(out=ident[0:64, :], in_=ident_in[:].bitcast(f32r))
        nc.sync.dma_start(out=ident[64:128, :], in_=ident_in[:].bitcast(f32r))
        ones_t = const_pool.tile([128, 64], f32r)
        nc.vector.memset(ones_t[:].bitcast(f32), 1.0)

        w_sb = {}
        for name, src in (("q", wq), ("k", wk), ("v", wv), ("o", wo)):
            t_ = const_pool.tile(
                [128, NCC, 128], f32r, tag=f"w{name}", name=f"w{name}"
            )
            nc.sync.dma_start(
                out=t_[:],
                in_=src.rearrange("(j p) d -> p j d", p=128).bitcast(f32r),
            )
            w_sb[name] = t_

        # persistent per-batch state, rotated via pools
        state = {}

        def phase_proj(b, tt):
            """QKV projection + V transpose for t-tile tt of batch b."""
            if tt == 0:
                state[b] = {
                    "q2t": qk_pool.tile([128, T], f32r, tag="q2t", name="q2t"),
                    "k2t": qk_pool.tile([128, T], f32r, tag="k2t", name="k2t"),
                    "vn": [
                        vn_pool.tile([128, NKC, 65], f32r, tag="vn", name="vn")
                        for _ in range(HPC)
                    ],
                }
                for h in range(HPC):
                    nc.vector.memset(state[b]["vn"][h][:, :, 64:65].bitcast(f32), 1.0)
            st = state[b]
            col0 = b * T + tt * TQ
            xt = stream_pool.tile([128, NCC, TQ], f32r, tag="xt", name="xt")
            nc.gpsimd.dma_start(
                out=xt[:],
                in_=xT[:, col0 : col0 + TQ]
                .rearrange("(j p) t -> p j t", p=128)
                .bitcast(f32r),
            )
            outs = {}
            order = ["q", "k", "v"]
            for idx, name in enumerate(order):
                ps = proj_ps.tile([128, TQ], f32, tag="proj", name="proj")
                for j in range(NCC):
                    nc.tensor.matmul(
                        ps[:, :],
                        w_sb[name][:, j, :],
                        xt[:, j, :],
                        start=(j == 0),
                        stop=(j == NCC - 1),
                    )
                outs[name] = ps
                # evacuate as soon as ready to free the slot for 3rd accum
                if name == "q":
                    nc.vector.tensor_copy(
                        out=st["q2t"][:, tt * TQ : (tt + 1) * TQ], in_=ps[:, :]
                    )
                elif name == "k":
                    nc.vector.tensor_copy(
                        out=st["k2t"][:, tt * TQ : (tt + 1) * TQ], in_=ps[:, :]
                    )
            v2t = v2t_pool.tile([128, TQ], f32r, tag="v2t", name="v2t")
            nc.vector.tensor_copy(out=v2t[:], in_=outs["v"][:, :])
            # transpose V^T [64,128] slices -> VN [128,64] chunks (4 per bank)
            kc0 = tt * (TQ // KC)
            for h in range(HPC):
                pt = proj_ps.tile([128, 512], f32, tag="proj", name="proj")
                for sc in range(TQ // KC):
                    nc.tensor.transpose(
                        pt[0:128, 64 * sc : 64 * sc + 64].bitcast(f32r),
                        v2t[64 * h : 64 * h + 64, sc * KC : (sc + 1) * KC],
                        ident[64 * h : 64 * h + 64, :],
                    )
                nc.vector.tensor_copy(
                    out=st["vn"][h][:, kc0 : kc0 + 4, 0:64],
                    in_=pt[0:128, 0:256].bitcast(f32r).rearrange(
                        "p (c d) -> p c d", d=64
                    ),
                )

        def phase_attn(b, qt):
            """Attention for q-tile qt of batch b (needs proj tiles <= qt)."""
            st = state[b]
            nk = (qt + 1) * (TQ // KC)  # causal k-chunks
            if qt == 0 and "attnall" not in st:
                st["attnall"] = attnall_pool.tile(
                    [128, T], f32, tag="attnall", name="attnall"
                )

            qsl = slice(qt * TQ, (qt + 1) * TQ)
            groups = [
                list(range(g, min(g + GROUP, nk))) for g in range(0, nk, GROUP)
            ]
            avp = {}
            for h in range(HPC):
                avp[h] = av_ps.tile([128, TQ], f32, tag="av", name="av")
            hs = slice(0, 64), slice(64, 128)

            # software pipeline: scores one group ahead of exp+av, per head
            pend = []  # (h, group, sc_tile)

            def flush_one():
                h, g, ps = pend.pop(0)
                et = e_pool.tile([128, GROUP * TQ], f32r, tag="etile", name="etile")
                # exp: coalesce full chunks, handle diag chunks separately
                run = []  # consecutive full-chunk local idxs

                def flush_run():
                    if not run:
                        return
                    j0, j1 = run[0], run[-1]
                    nc.scalar.activation(
                        out=et[:, j0 * TQ : (j1 + 1) * TQ],
                        in_=ps[:, j0 * TQ : (j1 + 1) * TQ],
                        func=mybir.ActivationFunctionType.Exp,
                        scale=0.125,
                    )
                    run.clear()

                for j, kc in enumerate(g):
                    o = kc * KC - qt * TQ  # diag offset
                    if o < 0:
                        run.append(j)
                        continue
                    flush_run()
                    if o > 0:
                        nc.vector.memset(et[:, j * TQ : j * TQ + o].bitcast(f32), 0.0)
                    nc.scalar.activation(
                        out=et[:, j * TQ + o : (j + 1) * TQ],
                        in_=ps[:, j * TQ + o : (j + 1) * TQ],
                        func=mybir.ActivationFunctionType.Exp,
                        scale=0.125,
                    )
                    nc.vector.tensor_mul(
                        et[:, j * TQ + o : j * TQ + o + 128],
                        et[:, j * TQ + o : j * TQ + o + 128],
                        band[:],
                    )
                flush_run()
                for j, kc in enumerate(g):
                    nc.tensor.matmul(
                        avp[h][0:65, :],
                        st["vn"][h][:, kc, :],
                        et[:, j * TQ : (j + 1) * TQ],
                        start=(kc == 0),
                        stop=(kc == nk - 1),
                    )

            for gi, g in enumerate(groups):
                for h in range(HPC):
                    ps = sc_ps.tile([128, GROUP * TQ], f32, tag="sc", name="sc")
                    for j, kc in enumerate(g):
                        nc.tensor.matmul(
                            ps[:, j * TQ : (j + 1) * TQ],
                            st["k2t"][hs[h], kc * KC : (kc + 1) * KC],
                            st["q2t"][hs[h], qsl],
                            start=True,
                            stop=True,
                        )
                    pend.append((h, g, ps))
                    while len(pend) > 2:
                        flush_one()
            while pend:
                flush_one()

            # per-q-tile normalization straight from psum
            g_t = gath_pool.tile([128, TQ], f32, tag="gather", name="gather")
            for h in range(HPC):
                nc.vector.tensor_copy(
                    out=g_t[32 * h : 32 * h + 1, :], in_=avp[h][64:65, :]
                )
            nc.vector.reciprocal(g_t[0:33, :], g_t[0:33, :])
            gr = gath_pool.tile([128, TQ], f32r, tag="gatr", name="gatr")
            nc.vector.tensor_copy(out=gr[0:33, :], in_=g_t[0:33, :])
            for h in range(HPC):
                slot = 32 * h
                rp = sc_ps.tile([128, GROUP * TQ], f32, tag="sc", name="sc")
                nc.tensor.matmul(
                    rp[0:64, 0:TQ],
                    ones_t[slot : slot + 1, :],
                    gr[slot : slot + 1, :],
                    start=True,
                    stop=True,
                    tile_position=(slot, 0),
                )
                rsb = gath_pool.tile([64, TQ], f32, tag="rsb", name="rsb")
                nc.vector.tensor_copy(out=rsb[:, :], in_=rp[0:64, 0:TQ])
                nc.vector.tensor_mul(
                    st["attnall"][hs[h], qt * TQ : (qt + 1) * TQ],
                    avp[h][0:64, :],
                    rsb[:, :],
                )

            if qt == NT - 1:
                nc.sync.dma_start(out=attn_dram[b][:], in_=st["attnall"][:])
                if MODEL_NO_COLLECTIVE:
                    nc.sync.dma_start(
                        out=ag_out[b][0:128, :], in_=attn_dram[b][:]
                    )
                else:
                    nc.gpsimd.collective_compute(
                        "AllGather",
                        mybir.AluOpType.bypass,
                        ins=[attn_dram[b][:]],
                        outs=[ag_out[b][:]],
                        replica_groups=[list(range(N_CORES))],
                    )

        def phase_out(b, tt):
            """Out-projection for t-tile tt of batch b (after AllGather b)."""
            col0 = b * T + tt * TQ
            halves = []
            for hh in range(2):
                agt = stream_pool.tile(
                    [128, NCC // 2, TQ], f32r, tag="agt", name="agt"
                )
                nc.sync.dma_start(
                    out=agt[:],
                    in_=ag_out[b][
                        hh * 512 : (hh + 1) * 512, tt * TQ : (tt + 1) * TQ
                    ]
                    .rearrange("(j p) t -> p j t", p=128)
                    .bitcast(f32r),
                )
                halves.append(agt)
            ps = proj_ps.tile([128, 512], f32, tag="proj", name="proj")
            for j in range(NCC):
                nc.tensor.matmul(
                    ps[:, 0:TQ],
                    w_sb["o"][:, j, :],
                    halves[j // 4][:, j % 4, :],
                    start=(j == 0),
                    stop=(j == NCC - 1),
                )
            osb = osb_pool.tile([128, TQ], f32, tag="osb", name="osb")
            nc.vector.tensor_copy(out=osb[:], in_=ps[:, 0:TQ])
            nc.sync.dma_start(out=outT[:, col0 : col0 + TQ], in_=osb[:])

        # ---- schedule: interleave proj(b,tt) and attn(b,qt=tt); outproj of
        # batch b-1 interleaves with batch b ----
        pending = []
        for b in range(B):
            for tt in range(NT):
                phase_proj(b, tt)
                phase_attn(b, tt)
                n_emit = 0 if tt == 0 else (2 if tt == NT - 1 else 1)
                for _ in range(min(n_emit, len(pending))):
                    phase_out(*pending.pop(0))
            pending.extend((b, tt) for tt in range(NT))
        for args in pending:
            phase_out(*args)

    _split_multi_waits(nc)
    return nc


def _prepare_inputs(x, W_qkv, W_out):
    xT = np.ascontiguousarray(
        x.reshape(B * T, D_MODEL).T, dtype=np.float32
    )
    band = (np.arange(128)[None, :] >= np.arange(128)[:, None]).astype(np.float32)
    ident = np.eye(64, dtype=np.float32)
    in_maps = []
    for i in range(N_CORES):
        hd = slice(128 * i, 128 * (i + 1))
        in_maps.append(
            {
                "xT": xT,
                "wq": np.ascontiguousarray(W_qkv[:, hd]),
                "wk": np.ascontiguousarray(W_qkv[:, D_MODEL:][:, hd]),
                "wv": np.ascontiguousarray(W_qkv[:, 2 * D_MODEL:][:, hd]),
                "wo": np.ascontiguousarray(W_out[:, hd]),
                "band": band,
                "ident": ident,
            }
        )
    return in_maps


def run(x, W_qkv, W_out, trace=False):
    import sys

    if "/opt/trn_rl_repo" not in sys.path:
        sys.path.insert(0, "/opt/trn_rl_repo")
    from concourse.bass_utils import run_bass_kernel_spmd

    key = "program"
    if key not in _PROGRAM_CACHE:
        _PROGRAM_CACHE[key] = _build_program()
    nc = _PROGRAM_CACHE[key]
    in_maps = _prepare_inputs(x, W_qkv, W_out)
    res = run_bass_kernel_spmd(
        nc, in_maps, core_ids=list(range(N_CORES)), trace=trace
    )
    outT_full = np.concatenate(
        [res.results[i]["outT"] for i in range(N_CORES)], axis=0
    )  # [1024, B*T]
    out = np.ascontiguousarray(outT_full.T).reshape(B, T, D_MODEL)
    return out, res


def kernel(x, W_qkv, W_out):
    out, _ = run(
        np.asarray(x, dtype=np.float32),
        np.asarray(W_qkv, dtype=np.float32),
        np.asarray(W_out, dtype=np.float32),
    )
    return out



# revision 11
# speedup vs baseline: 1.6649x; 1.6649x over previous
"""Causal self-attention on 8 trn2 NeuronCores.

Sharding v2: (batch, head-half) per core. Core c handles batch b=c//2 and
heads hh*8..hh*8+7 where hh=c%2. QKV projection + attention run fully
local in bf16; the two cores of a batch exchange their attention-output
halves with a pair AllGather (staged per 512-token q-tile so the exchange
overlaps later q-tiles); out-projection is column-parallel within the
pair (512 output cols/core); host concat + transpose.

Layout per core (pairs p=0..3, local heads 2p, 2p+1):
  Q2T/K2T[p] [128, T] bf16  transposed q/k, head 2p dims on partitions
                            0:64, head 2p+1 on 64:128.
  VN[head]   [128, 16, 65] bf16 per-head V k-chunks + ones column
                            (column 64) so the AV matmul also emits the
                            softmax row-sum at psum partition 64.
  scores     S^T chunk [128 k, <=512 q] f32 psum; diagonal chunks are
             computed sliced ([o:512]) instead of masked+memset.
  E^T        exp(S/8) bf16 via ACT from psum; 128-wide band mask mult on
             the diagonal block only.
  attnall[p] [128, T] bf16, normalized via reciprocal_approx_fast +
             ones-outer-product broadcast matmul.
"""

import numpy as np

D_MODEL = 1024
N_HEADS = 16
HEAD_DIM = 64
B = 4
T = 2048
N_CORES = 8
TQ = 512          # q tile
KC = 128          # k chunk
GROUP = 2         # k-chunks per exp group
NT = T // TQ      # q tiles per batch (4)
NKC = T // KC     # k chunks per batch (16)
NCC = D_MODEL // 128  # contraction chunks (8)
NPAIR = 4         # head pairs per core (8 heads)
MODEL_NO_COLLECTIVE = False  # timing-analysis only: swap AG for local DMA

_PROGRAM_CACHE = {}


def _split_multi_waits(nc, max_waits=1):
    """This toolchain's walrus encodes at most one sync-wait per
    instruction; hoist excess waits onto same-engine carrier nops."""
    import concourse.mybir as mybir

    ctr = 0
    for f in nc.m.functions:
        new_blocks = []
        for bb in f.blocks:
            insts = list(bb.instructions)
            if not any(
                inst.sync_info is not None and len(inst.sync_info.on_wait) > max_waits
                for inst in insts
            ):
                new_blocks.append(bb)
                continue
            out = []
            for inst in insts:
                si = inst.sync_info
                if si is not None and len(si.on_wait) > max_waits:
                    waits = list(si.on_wait)
                    excess = waits[max_waits:]
                    while excess:
                        ctr += 1
                        nop = mybir.InstNoOp(
                            name=f"waitcarrier-{ctr}", engine=inst.engine
                        )
                        nop.sync_info = mybir.SyncInfo(
                            on_wait=excess[:max_waits], on_update=[]
                        )
                        out.append(nop)
                        excess = excess[max_waits:]
                    si.on_wait = waits[:max_waits]
                out.append(inst)
            nb = mybir.BasicBlock(
                name=bb.name,
                instructions=[],
                IsPredicated=bb.IsPredicated,
                IsExit=bb.IsExit,
                IsLoopEntry=bb.IsLoopEntry,
            )
            for i in out:
                nb.add_instruction(i)
            new_blocks.append(nb)
        f.blocks = new_blocks


def _build_program():
    import concourse.bass as bass
    import concourse.mybir as mybir
    import concourse.tile as tile
    from contextlib import ExitStack

    f32 = mybir.dt.float32
    bf16 = mybir.dt.bfloat16
    nc = bass.Bass()

    xT = nc.declare_dram_parameter("xT", [D_MODEL, T], bf16, isOutput=False)
    wq = nc.declare_dram_parameter("wq", [D_MODEL, TQ], bf16, isOutput=False)
    wk = nc.declare_dram_parameter("wk", [D_MODEL, TQ], bf16, isOutput=False)
    wv = nc.declare_dram_parameter("wv", [D_MODEL, TQ], bf16, isOutput=False)
    wo = nc.declare_dram_parameter("wo", [D_MODEL, TQ], bf16, isOutput=False)
    band_in = nc.declare_dram_parameter("band", [128, 128], bf16, isOutput=False)
    ident_in = nc.declare_dram_parameter("ident", [64, 64], bf16, isOutput=False)
    outT = nc.declare_dram_parameter("outT", [TQ, T], f32, isOutput=True)

    attn_dram = [nc.dram_tensor(f"attn_d{qt}", [512, TQ], bf16) for qt in range(NT)]
    ag_out = [
        nc.dram_tensor(f"ag_out{qt}", [1024, TQ], bf16) for qt in range(NT)
    ]
    PAIRS = [[0, 1], [2, 3], [4, 5], [6, 7]]

    with tile.TileContext(nc) as tc, ExitStack() as ctx:
        const_pool = ctx.enter_context(tc.tile_pool(name="const", bufs=1))
        qk_pool = ctx.enter_context(tc.tile_pool(name="qk", bufs=1))
        vn_pool = ctx.enter_context(tc.tile_pool(name="vn", bufs=1))
        stream_pool = ctx.enter_context(tc.tile_pool(name="stream", bufs=2))
        v2t_pool = ctx.enter_context(tc.tile_pool(name="v2t", bufs=2))
        e_pool = ctx.enter_context(tc.tile_pool(name="etile", bufs=4))
        gath_pool = ctx.enter_context(tc.tile_pool(name="gath", bufs=3))
        attnall_pool = ctx.enter_context(tc.tile_pool(name="attnall", bufs=1))
        osb_pool = ctx.enter_context(tc.tile_pool(name="osb", bufs=2))
        proj_ps = ctx.enter_context(tc.tile_pool(name="projps", bufs=2, space="PSUM"))
        sc_ps = ctx.enter_context(tc.tile_pool(name="scps", bufs=2, space="PSUM"))
        av_ps = ctx.enter_context(tc.tile_pool(name="avps", bufs=2, space="PSUM"))

        # ---- constants / weights ----
        band = const_pool.tile([128, 128], bf16)
        nc.sync.dma_start(out=band[:], in_=band_in[:])
        ident = const_pool.tile([128, 64], bf16)
        nc.sync.dma_start(out=ident[0:64, :], in_=ident_in[:])
        nc.sync.dma_start(out=ident[64:128, :], in_=ident_in[:])
        ones_t = const_pool.tile([128, 64], bf16)
        nc.vector.memset(ones_t[:], 1.0)

        w_sb = {}
        for name, src in (("q", wq), ("k", wk), ("v", wv), ("o", wo)):
            t_ = const_pool.tile([128, NCC, TQ], bf16, tag=f"w{name}", name=f"w{name}")
            nc.sync.dma_start(
                out=t_[:], in_=src.rearrange("(j p) d -> p j d", p=128)
            )
            w_sb[name] = t_

        # persistent tiles (one batch per core)
        q2t = [
            qk_pool.tile([128, T], bf16, tag=f"q2t{p}", name=f"q2t{p}")
            for p in range(NPAIR)
        ]
        k2t = [
            qk_pool.tile([128, T], bf16, tag=f"k2t{p}", name=f"k2t{p}")
            for p in range(NPAIR)
        ]
        vn = [
            vn_pool.tile([128, NKC, 65], bf16, tag=f"vn{h}", name=f"vn{h}")
            for h in range(2 * NPAIR)
        ]
        for h in range(2 * NPAIR):
            nc.vector.memset(vn[h][:, :, 64:65], 1.0)
        attnall = [
            attnall_pool.tile([128, T], bf16, tag=f"attnall{p}", name=f"attnall{p}")
            for p in range(NPAIR)
        ]

        def phase_proj(tt):
            """QKV projection + V transpose for t-tile tt (all 4 pairs)."""
            col0 = tt * TQ
            xt = stream_pool.tile([128, NCC, TQ], bf16, tag="xt", name="xt")
            nc.gpsimd.dma_start(
                out=xt[:],
                in_=xT[:, col0 : col0 + TQ].rearrange("(j p) t -> p j t", p=128),
            )
            for p in range(NPAIR):
                cs = slice(p * 128, (p + 1) * 128)
                for name in ("q", "k"):
                    ps = proj_ps.tile([128, TQ], f32, tag="proj", name="proj")
                    for j in range(NCC):
                        nc.tensor.matmul(
                            ps[:, :],
                            w_sb[name][:, j, cs],
                            xt[:, j, :],
                            start=(j == 0),
                            stop=(j == NCC - 1),
                        )
                    dst = q2t[p] if name == "q" else k2t[p]
                    nc.vector.tensor_copy(
                        out=dst[:, col0 : col0 + TQ], in_=ps[:, :]
                    )
                # v chunk for this pair
                ps = proj_ps.tile([128, TQ], f32, tag="proj", name="proj")
                for j in range(NCC):
                    nc.tensor.matmul(
                        ps[:, :],
                        w_sb["v"][:, j, cs],
                        xt[:, j, :],
                        start=(j == 0),
                        stop=(j == NCC - 1),
                    )
                v2t = v2t_pool.tile([128, TQ], bf16, tag="v2t", name="v2t")
                nc.vector.tensor_copy(out=v2t[:], in_=ps[:, :])
                # transpose V^T [64,128] slices -> VN [128,64] chunks
                kc0 = tt * (TQ // KC)
                for h in range(2):
                    pt = proj_ps.tile([128, TQ], f32, tag="proj", name="proj")
                    for sc in range(TQ // KC):
                        nc.tensor.transpose(
                            pt[0:128, 32 * sc : 32 * sc + 32].bitcast(bf16),
                            v2t[64 * h : 64 * h + 64, sc * KC : (sc + 1) * KC],
                            ident[64 * h : 64 * h + 64, :],
                        )
                    nc.vector.tensor_copy(
                        out=vn[2 * p + h][:, kc0 : kc0 + 4, 0:64],
                        in_=pt[0:128, 0:128]
                        .bitcast(bf16)
                        .rearrange("p (c d) -> p c d", d=64),
                    )

        def phase_attn(p, qt, g_t):
            """Attention for q-tile qt, head pair p. Leaves unnormalized
            attn in attnall[p] and row-sums in g_t[h] (partition 32p)."""
            nk = (qt + 1) * (TQ // KC)  # causal k-chunks
            qsl = slice(qt * TQ, (qt + 1) * TQ)
            groups = [list(range(g, min(g + GROUP, nk))) for g in range(0, nk, GROUP)]
            avp = {}
            for h in range(2):
                avp[h] = av_ps.tile([128, TQ], f32, tag="av", name="av")
            hs = slice(0, 64), slice(64, 128)

            # software pipeline: scores one group ahead of exp+av, per head
            pend = []  # (h, group, sc_tile)

            def flush_one():
                h, g, ps = pend.pop(0)
                et = e_pool.tile([128, GROUP * TQ], bf16, tag="etile", name="etile")
                run = []  # consecutive full-chunk local idxs

                def flush_run():
                    if not run:
                        return
                    j0, j1 = run[0], run[-1]
                    nc.scalar.activation(
                        out=et[:, j0 * TQ : (j1 + 1) * TQ],
                        in_=ps[:, j0 * TQ : (j1 + 1) * TQ],
                        func=mybir.ActivationFunctionType.Exp,
                        scale=0.125,
                    )
                    run.clear()

                for j, kc in enumerate(g):
                    o = kc * KC - qt * TQ  # diag offset
                    if o < 0:
                        run.append(j)
                        continue
                    flush_run()
                    nc.scalar.activation(
                        out=et[:, j * TQ + o : (j + 1) * TQ],
                        in_=ps[:, j * TQ + o : (j + 1) * TQ],
                        func=mybir.ActivationFunctionType.Exp,
                        scale=0.125,
                    )
                    nc.vector.tensor_mul(
                        et[:, j * TQ + o : j * TQ + o + 128],
                        et[:, j * TQ + o : j * TQ + o + 128],
                        band[:],
                    )
                flush_run()
                for j, kc in enumerate(g):
                    o = max(0, kc * KC - qt * TQ)
                    nc.tensor.matmul(
                        avp[h][0:65, o:TQ],
                        vn[2 * p + h][:, kc, :],
                        et[:, j * TQ + o : (j + 1) * TQ],
                        start=(kc == 0),
                        stop=(kc == nk - 1),
                    )

            for gi, g in enumerate(groups):
                for h in range(2):
                    ps = sc_ps.tile([128, GROUP * TQ], f32, tag="sc", name="sc")
                    for j, kc in enumerate(g):
                        o = max(0, kc * KC - qt * TQ)
                        nc.tensor.matmul(
                            ps[:, j * TQ + o : (j + 1) * TQ],
                            k2t[p][hs[h], kc * KC : (kc + 1) * KC],
                            q2t[p][hs[h], qt * TQ + o : (qt + 1) * TQ],
                            start=True,
                            stop=True,
                        )
                    pend.append((h, g, ps))
                    while len(pend) > 2:
                        flush_one()
            while pend:
                flush_one()

            # evacuate unnormalized attn + row-sums; free psum asap
            for h in range(2):
                nc.vector.tensor_copy(
                    out=g_t[h][32 * p : 32 * p + 1, :], in_=avp[h][64:65, :]
                )
                nc.vector.tensor_copy(
                    out=attnall[p][hs[h], qsl], in_=avp[h][0:64, :]
                )

        def phase_norm(qt, g_t):
            """Batched softmax normalization for all 4 pairs of q-tile qt."""
            qsl = slice(qt * TQ, (qt + 1) * TQ)
            hs = slice(0, 64), slice(64, 128)
            gr = {}
            for h in range(2):
                nc.vector.reciprocal(g_t[h][0:97, :], g_t[h][0:97, :])
                gr[h] = gath_pool.tile([128, TQ], bf16, tag="gatr", name="gatr")
                nc.vector.tensor_copy(out=gr[h][0:97, :], in_=g_t[h][0:97, :])
            for p in range(NPAIR):
                slot = 32 * p
                rp = sc_ps.tile([128, GROUP * TQ], f32, tag="sc", name="sc")
                for h in range(2):
                    nc.tensor.matmul(
                        rp[64 * h : 64 * h + 64, 0:TQ],
                        ones_t[slot : slot + 1, :],
                        gr[h][slot : slot + 1, :],
                        start=True,
                        stop=True,
                        tile_position=(slot, 64 * h),
                    )
                rsb = gath_pool.tile([128, TQ], bf16, tag="rsb", name="rsb")
                nc.vector.tensor_copy(out=rsb[:, :], in_=rp[0:128, 0:TQ])
                nc.vector.tensor_mul(
                    attnall[p][:, qsl],
                    attnall[p][:, qsl],
                    rsb[:, :],
                )

        def stage_ag(qt):
            qsl = slice(qt * TQ, (qt + 1) * TQ)
            for p in range(NPAIR):
                nc.sync.dma_start(
                    out=attn_dram[qt][p * 128 : (p + 1) * 128, :],
                    in_=attnall[p][:, qsl],
                )
            if MODEL_NO_COLLECTIVE:
                nc.sync.dma_start(out=ag_out[qt][0:512, :], in_=attn_dram[qt][:])
                nc.sync.dma_start(out=ag_out[qt][512:1024, :], in_=attn_dram[qt][:])
            else:
                nc.gpsimd.collective_compute(
                    "AllGather",
                    mybir.AluOpType.bypass,
                    ins=[attn_dram[qt][:]],
                    outs=[ag_out[qt][:]],
                    replica_groups=PAIRS,
                )

        def phase_out(qt):
            """Out-projection for q-tile qt (after pair AllGather qt)."""
            agt = stream_pool.tile([128, NCC, TQ], bf16, tag="agt", name="agt")
            nc.sync.dma_start(
                out=agt[:],
                in_=ag_out[qt][:].rearrange("(j p) t -> p j t", p=128),
            )
            for c4 in range(4):
                cs = slice(c4 * 128, (c4 + 1) * 128)
                ps = proj_ps.tile([128, TQ], f32, tag="proj", name="proj")
                for j in range(NCC):
                    nc.tensor.matmul(
                        ps[:, :],
                        w_sb["o"][:, j, cs],
                        agt[:, j, :],
                        start=(j == 0),
                        stop=(j == NCC - 1),
                    )
                osb = osb_pool.tile([128, TQ], f32, tag="osb", name="osb")
                nc.vector.tensor_copy(out=osb[:], in_=ps[:, :])
                nc.sync.dma_start(
                    out=outT[cs, qt * TQ : (qt + 1) * TQ], in_=osb[:]
                )

        # ---- schedule ----
        for tt in range(NT):
            phase_proj(tt)
            g_t = {
                h: gath_pool.tile([128, TQ], f32, tag=f"gather{h}", name="gather")
                for h in range(2)
            }
            for p in range(NPAIR):
                phase_attn(p, tt, g_t)
            phase_norm(tt, g_t)
            stage_ag(tt)
            if tt > 0:
                phase_out(tt - 1)
        phase_out(NT - 1)

    _split_multi_waits(nc)
    return nc


def _prepare_inputs(x, W_qkv, W_out):
    import ml_dtypes

    bf16 = ml_dtypes.bfloat16
    band = (np.arange(128)[None, :] >= np.arange(128)[:, None]).astype(bf16)
    ident = np.eye(64, dtype=bf16)
    Wq = W_qkv[:, 0:D_MODEL]
    Wk = W_qkv[:, D_MODEL : 2 * D_MODEL]
    Wv = W_qkv[:, 2 * D_MODEL :]
    in_maps = []
    for c in range(N_CORES):
        b, hh = c // 2, c % 2
        hd = slice(512 * hh, 512 * (hh + 1))
        in_maps.append(
            {
                "xT": np.ascontiguousarray(x[b].T).astype(bf16),
                "wq": np.ascontiguousarray(Wq[:, hd]).astype(bf16),
                "wk": np.ascontiguousarray(Wk[:, hd]).astype(bf16),
                "wv": np.ascontiguousarray(Wv[:, hd]).astype(bf16),
                "wo": np.ascontiguousarray(W_out[:, hd]).astype(bf16),
                "band": band,
                "ident": ident,
            }
        )
    return in_maps


def run(x, W_qkv, W_out, trace=False):
    import sys

    if "/opt/trn_rl_repo" not in sys.path:
        sys.path.insert(0, "/opt/trn_rl_repo")
    from concourse.bass_utils import run_bass_kernel_spmd

    key = "program"
    if key not in _PROGRAM_CACHE:
        _PROGRAM_CACHE[key] = _build_program()
    nc = _PROGRAM_CACHE[key]
    in_maps = _prepare_inputs(x, W_qkv, W_out)
    res = run_bass_kernel_spmd(
        nc, in_maps, core_ids=list(range(N_CORES)), trace=trace
    )
    out = np.empty((B, T, D_MODEL), dtype=np.float32)
    for c in range(N_CORES):
        b, hh = c // 2, c % 2
        out[b, :, 512 * hh : 512 * (hh + 1)] = res.results[c]["outT"].T
    return out, res


def kernel(x, W_qkv, W_out):
    out, _ = run(
        np.asarray(x, dtype=np.float32),
        np.asarray(W_qkv, dtype=np.float32),
        np.asarray(W_out, dtype=np.float32),
    )
    return out
